# revision 1
# baseline (speedup 1.0000x reference)
"""Trainium2 Bass kernel for nn_Bottleneck_57561151701110 (SAM pairwise
bottleneck block). Data-parallel over batch: 8 images -> 8 NeuronCores.

v2: all remaps are SBUF->SBUF DMAs (no DRAM scratch roundtrips), x stays
resident for the residual, leaky-relu fused into ACT Lrelu, softmax
reciprocal via ACT ln/exp, conv biases folded into ACT bias operands so
the PE stream is pure matmuls, consts packed into 3 DMAs.

Per-core pipeline (one 256x56x56 image):
  conv1/2 (PE f32r) -> x12s bf16 -> band remap (DMA) ->
  conv3 (PE f32r) -> x3sfull bf16 (58x58 zero-border planes) -> xg remap (DMA)
  per k: feat = x1b - x2b[window] (DVE bf16 2x) -> relu (DVE 4x) ->
    w1/w2 blockdiag matmuls (PE bf16) -> hp relu / e exp (ACT) -> Z (PE)
  rz = exp(-ln(Z)) (ACT) -> agg: pk = xg*e (DVE 2x), k-sum (PE identity MM),
  Lrelu (ACT), *rz (DVE) -> sqfull -> sam remap (DMA) -> convo (PE) ->
  Lrelu+bias (ACT) -> +x residual (DVE) -> out.
"""

import os
import sys

for _p in ("/opt/trn_rl_repo", os.path.expanduser("~/.axon_site/_ro/trn_rl_repo")):
    if os.path.isdir(_p) and _p not in sys.path:
        sys.path.insert(0, _p)

from contextlib import ExitStack

import numpy as np

import concourse.bass as bass
import concourse.bacc as bacc
import concourse.tile as tile
from concourse import mybir
from concourse.bass_utils import run_bass_kernel_spmd

dt = mybir.dt
ALU = mybir.AluOpType
ACTF = mybir.ActivationFunctionType

B, CIN, H, W = 8, 256, 56, 56
NPIX = H * W            # 3136
REL, MID, OUT = 32, 256, 256
SHARE = 8
NB = 4                  # row bands
BH = H // NB            # 14 rows per band
Q = BH * W              # 784 band pixels
NEG = 0.01
BN_EPS = 1e-5
CCH = 448               # conv free chunk (8 rows)
NCH = NPIX // CCH       # 7
OFFS = [(dh, dw) for dh in (-1, 0, 1) for dw in (-1, 0, 1)]
# phase-C processing order: dw=0 offsets last (they need the late x2bB copy)
OFFS_ORD = [0, 2, 3, 5, 6, 8, 1, 4, 7]
NSQ = SHARE // 2        # 4 s per quad
HBW = Q // 2            # 392 half-band pixels (7 rows)

F32, F32R, BF16 = dt.float32, dt.float32r, dt.float16  # 16-bit = fp16 (values < 16)

_CACHE = {}

# packed const layouts (cols)
#  cf (f32, 128 part):  c12 (2*64) | c3 (2*256) | vecs (8)
CF_C12, CF_C3, CF_VEC = 0, 128, 128 + 512
CF_COLS = CF_VEC + 8
#  cb (bf16, 128 part): co (2*256) | w1 (128) | w2 (128) | ident (128)
CB_CO, CB_W1, CB_W2, CB_ID = 0, 512, 640, 768
CB_COLS = 896
#  pb (bf16, 8 part): post (128) | posr (9*784)
PB_POST, PB_POSR = 0, 128
PB_COLS = 128 + 9 * Q


# ----------------------------------------------------------------- host prep
def _position(h, w):
    loc_w = np.tile(np.linspace(-1.0, 1.0, w, dtype=np.float32)[None, :], (h, 1))
    loc_h = np.tile(np.linspace(-1.0, 1.0, h, dtype=np.float32)[:, None], (1, w))
    return np.stack([loc_w, loc_h], axis=0)  # (2, H, W)


def _host_consts(inp):
    f32 = np.float32
    bf16 = np.float16
    inv_a = (inp["bna_g"] / np.sqrt(inp["bna_v"] + BN_EPS)).astype(f32)
    beta_a = (inp["bna_b"] - inp["bna_m"] * inv_a).astype(f32)
    inv_b = (inp["bnb_g"] / np.sqrt(inp["bnb_v"] + BN_EPS)).astype(f32)
    beta_b = (inp["bnb_b"] - inp["bnb_m"] * inv_b).astype(f32)

    w1c = inp["conv1_w"] * inv_a[:REL, None]
    b1 = inp["conv1_b"] * inv_a[:REL] + beta_a[:REL]
    w2c = inp["conv2_w"] * inv_a[:REL, None]
    b2 = inp["conv2_b"] * inv_a[:REL]

    # conv1+conv2 fused stationary: (256, 64), chunked over K
    c12 = np.concatenate([w1c, w2c], axis=0).T.astype(f32)  # (256,64)
    c3 = inp["conv3_w"].T.astype(f32)    # (256,256) lhsT

    # W1' with bnb scale folded
    w1p = (inp["w1"] * inv_b[:, None]).astype(f32)  # (32, 34)
    w1a, w1b = w1p[:, :REL], w1p[:, REL:]
    lhsT_w1 = np.zeros((128, 128), f32)
    lhsT_pos = np.zeros((8, 128), f32)
    lhsT_w2 = np.zeros((128, 128), f32)
    for b in range(NB):
        lhsT_w1[32 * b:32 * b + 32, 32 * b:32 * b + 32] = w1a.T  # [c, o]
        lhsT_pos[2 * b:2 * b + 2, 32 * b:32 * b + 32] = w1b.T    # [c2, o]
        lhsT_w2[32 * b:32 * b + 32, 32 * b:32 * b + 32] = inp["w2"].T  # [o, g]

    # position branch, batch independent: posr[2b+c2, 784k+q] =
    # relu(inv_a[32+c2]*subp[c2,k,band b pix q] + beta_a[32+c2])
    pos = _position(H, W)
    pc = np.einsum("oc,chw->ohw", inp["convp_w"], pos) + inp["convp_b"][:, None, None]
    pcp = np.pad(pc, ((0, 0), (1, 1), (1, 1)))
    posr = np.zeros((8, 9 * Q), f32)
    for k, (dh, dw) in enumerate(OFFS):
        sub = pc - pcp[:, 1 + dh:1 + dh + H, 1 + dw:1 + dw + W]  # (2,56,56)
        v = np.maximum(inv_a[REL:, None, None] * sub + beta_a[REL:, None, None], 0.0)
        vb = v.reshape(2, NB, BH, W)  # (c2, b, r, w)
        for b in range(NB):
            posr[2 * b:2 * b + 2, Q * k:Q * (k + 1)] = vb[:, b].reshape(2, Q)

    # vecs: per-partition ACT bias columns
    #  0: bias12 (64)   1: conv3_b[:128]  2: conv3_b[128:]
    #  3: betab (128)   4: w2bv (128)     5: biaso[:128]   6: biaso[128:]
    vecs = np.zeros((128, 8), f32)
    vecs[:REL, 0] = b1
    vecs[REL:2 * REL, 0] = b2
    vecs[:, 1] = inp["conv3_b"][:128]
    vecs[:, 2] = inp["conv3_b"][128:]
    vecs[:, 3] = np.tile(beta_b, NB)
    vecs[:, 4] = np.tile(inp["w2_b"], NB)
    vecs[:, 5] = inp["convo_b"][:128]
    vecs[:, 6] = inp["convo_b"][128:]

    cf = np.zeros((128, CF_COLS), f32)
    cf[:, CF_C12:CF_C12 + 64] = c12[:128]
    cf[:, CF_C12 + 64:CF_C12 + 128] = c12[128:]
    cf[:, CF_C3:CF_C3 + 256] = c3[:128]
    cf[:, CF_C3 + 256:CF_C3 + 512] = c3[128:]
    cf[:, CF_VEC:CF_VEC + 8] = vecs

    co = inp["convo_w"].T.astype(bf16)   # (256,256) lhsT
    cb = np.zeros((128, CB_COLS), bf16)
    cb[:, CB_CO:CB_CO + 256] = co[:128]
    cb[:, CB_CO + 256:CB_CO + 512] = co[128:]
    cb[:, CB_W1:CB_W1 + 128] = lhsT_w1.astype(bf16)
    cb[:, CB_W2:CB_W2 + 128] = lhsT_w2.astype(bf16)
    cb[:, CB_ID:CB_ID + 128] = np.eye(128, dtype=bf16)

    pb = np.zeros((8, PB_COLS), bf16)
    pb[:, PB_POST:PB_POST + 128] = lhsT_pos.astype(bf16)
    pb[:, PB_POSR:] = posr.astype(bf16)

    return {"cf": cf, "cb": cb, "pb": pb}


DEBUG = os.environ.get("KDEBUG", "0") == "1"


# ------------------------------------------------------------ program build
def _build_program():
    nc = bacc.Bacc("TRN2", target_bir_lowering=False, debug=False,
                   enable_asserts=False, num_devices=8)

    xin = nc.dram_tensor("xin", [CIN, NPIX], F32R, kind="ExternalInput").ap()
    cfd = nc.dram_tensor("cf", [128, CF_COLS], F32R, kind="ExternalInput").ap()
    cbd = nc.dram_tensor("cb", [128, CB_COLS], BF16, kind="ExternalInput").ap()
    pbd = nc.dram_tensor("pb", [8, PB_COLS], BF16, kind="ExternalInput").ap()
    outd = nc.dram_tensor("out", [CIN, NPIX], F32, kind="ExternalOutput").ap()
    dbg = {}
    if DEBUG:
        for nm, shape in [("d_x12s", [64, NPIX]), ("d_x1b", [128, Q]),
                          ("d_x2b", [128, 16 * 58]), ("d_e0", [128, Q]),
                          ("d_e4", [128, Q]), ("d_rz", [128, Q]),
                          ("d_sqf", [128, SHARE * Q]),
                          ("d_samsb", [128, 2 * NPIX]),
                          ("d_xga", [128, SHARE * 16 * 58])]:
            dbg[nm] = nc.dram_tensor(nm, shape, BF16, kind="ExternalOutput").ap()

    with tile.TileContext(nc) as tc, ExitStack() as ctx:
        nc_ = tc.nc

        # ---- persistent pools
        cpool = ctx.enter_context(tc.tile_pool(name="consts", bufs=1))
        xpool = ctx.enter_context(tc.tile_pool(name="xin", bufs=1))
        sbp = ctx.enter_context(tc.tile_pool(name="sbp", bufs=1))
        epool = ctx.enter_context(tc.tile_pool(name="epool", bufs=9))

        # ---- const loads: 3 packed DMAs on the sync queue
        # consts on the scalar queue so the sync queue starts x immediately
        cft = cpool.tile([128, CF_COLS], F32R, tag="cf")
        nc.scalar.dma_start(cft[:], cfd[:])
        cbt = cpool.tile([128, CB_COLS], BF16, tag="cb")
        nc.scalar.dma_start(cbt[:], cbd[:])
        pbt = cpool.tile([8, PB_COLS], BF16, tag="pb")
        nc.scalar.dma_start(pbt[:], pbd[:])

        c12t = cft[:, CF_C12:CF_C12 + 128].rearrange("p (k c) -> p k c", k=2)
        c3t = cft[:, CF_C3:CF_C3 + 512].rearrange("p (k c) -> p k c", k=2)
        vecst = cft[:, CF_VEC:CF_VEC + 8].bitcast(F32)
        cot = cbt[:, CB_CO:CB_CO + 512].rearrange("p (k c) -> p k c", k=2)
        w1t = cbt[:, CB_W1:CB_W1 + 128]
        w2t = cbt[:, CB_W2:CB_W2 + 128]
        identt = cbt[:, CB_ID:CB_ID + 128]
        post = pbt[:, PB_POST:PB_POST + 128]
        prt = pbt[:, PB_POSR:].rearrange("p (k q) -> p k q", k=9)

        # ---- x input: one resident tile, loaded in 14 chunk-slices (sync q)
        xfull = xpool.tile([128, 2, NPIX], F32R, tag="xfull")
        for c in range(NCH):
            for t in range(2):
                nc.sync.dma_start(
                    xfull[:, t, CCH * c:CCH * (c + 1)],
                    xin[128 * t:128 * (t + 1), CCH * c:CCH * (c + 1)])

        # ---- band-layout tiles + zero borders (memsets on idle GpSimd)
        x12s = sbp.tile([64, NPIX], BF16, tag="x12s")
        x1b = sbp.tile([128, NB * Q // 4], BF16, tag="x1b")     # [128, 784]
        x2b = sbp.tile([128, 16, 58], BF16, tag="x2b")
        x2bB = sbp.tile([128, 16, 58], BF16, tag="x2bB")
        x3sf = sbp.tile([128, 2, 58, 58], BF16, tag="x3sf")
        xgA = sbp.tile([128, SHARE, 16, 58], BF16, tag="xgA")
        xgB = sbp.tile([128, SHARE, 16, 58], BF16, tag="xgB")
        nc_.gpsimd.memset(x2b[:], 0.0)
        nc_.gpsimd.memset(x2bB[:, 0:1, 0:2], 0.0)
        nc_.gpsimd.memset(xgB[:, 0:1, 0:1, 0:1], 0.0)
        for t in range(2):
            nc_.gpsimd.memset(x3sf[:, t, 0:1, :], 0.0)      # top border row
            nc_.gpsimd.memset(x3sf[:, t, 57:58, :], 0.0)    # bottom border row
            nc_.gpsimd.memset(x3sf[:, t, 1:57, 0:1], 0.0)   # left border col
            nc_.gpsimd.memset(x3sf[:, t, 1:57, 57:58], 0.0)  # right border col

        # x12 chunk -> completed bands (band b rows 14b-1..14b+15 need chunks
        # floor((14b-1)/8)..ceil((14b+16)/8)-1)
        band_last_chunk = [1, 3, 5, 6]

        def emit_band_remap(b):
            # x1b: [32, 784] straight partition-offset copy
            nc.scalar.dma_start(x1b[32 * b:32 * b + 32, :],
                                x12s[0:REL, Q * b:Q * (b + 1)])
            # x2b: rows rlo..rhi of the 16-row halo window, cols 1..57
            rlo = 1 if b == 0 else 0
            rhi = 15 if b == NB - 1 else 16
            p0 = (BH * b - 1 + rlo) * W
            nc.scalar.dma_start(
                x2b[32 * b:32 * b + 32, rlo:rhi, 1:57],
                x12s[REL:2 * REL, p0:p0 + (rhi - rlo) * W]
                .rearrange("p (r w) -> p r w", w=W))

        # ---- warm the exp table set while the DMA queues are empty
        wpool = ctx.enter_context(tc.tile_pool(name="wpool", bufs=1))
        wsrc = wpool.tile([1, 8], F32, tag="wsrc")
        nc_.gpsimd.memset(wsrc[:], 0.0)
        wdst = wpool.tile([1, 8], F32, tag="wdst")
        nc_.scalar.activation(wdst[:], wsrc[:], ACTF.Exp)
        # ---- warm the PE HAM clock gate with dummy matmuls during the
        # input-DMA wait (idle PE defaults to 1.2 GHz; ~3.4us of activity
        # unthrottles it to 2.4 GHz before conv12 starts)
        with tc.tile_pool(name="pwarm", bufs=1, space="PSUM") as pwarm:
            wps_ = pwarm.tile([128, 128], F32, tag="wps")
            for _ in range(48):
                nc_.tensor.matmul(wps_[:], identt[:], identt[:],
                                  start=True, stop=True)

        # ---- phase A: conv12, all chunks first so band remaps issue early
        # (conv3 matmuls then fill the PE while the remap DMAs land)
        pscope1 = ExitStack()
        ppc = pscope1.enter_context(tc.tile_pool(name="ppc", bufs=2, space="PSUM"))
        for c in range(NCH):
            sl = slice(CCH * c, CCH * (c + 1))
            ps12 = ppc.tile([128, CCH], F32, tag="conv")
            nc_.tensor.matmul(ps12[0:64], c12t[0:128, 0, 0:64], xfull[:, 0, sl],
                              start=True, stop=False)
            nc_.tensor.matmul(ps12[0:64], c12t[0:128, 1, 0:64], xfull[:, 1, sl],
                              start=False, stop=True)
            nc_.scalar.activation(x12s[:, sl], ps12[0:64], ACTF.Identity,
                                  bias=vecst[0:64, 0:1])
            for b in range(NB):
                if band_last_chunk[b] == c:
                    emit_band_remap(b)
        # x2bB: x2b shifted one element right (flat), for dw=0 4B alignment
        nc.scalar.dma_start(
            x2bB[:].rearrange("p r w -> p (r w)")[:, 1:],
            x2b[:].rearrange("p r w -> p (r w)")[:, 0:16 * 58 - 1])

        # ---- phase B: conv3 (evictions split ACT/DVE; xg remaps per band)
        for c in range(NCH):
            sl = slice(CCH * c, CCH * (c + 1))
            for t in range(2):
                ps3 = ppc.tile([128, CCH], F32, tag="conv")
                nc_.tensor.matmul(ps3[:], c3t[:, 0, 128 * t:128 * (t + 1)],
                                  xfull[:, 0, sl], start=True, stop=False)
                nc_.tensor.matmul(ps3[:], c3t[:, 1, 128 * t:128 * (t + 1)],
                                  xfull[:, 1, sl], start=False, stop=True)
                dst3 = x3sf[:, t, 1 + 8 * c:1 + 8 * c + 8, 1:57]
                src3 = ps3[:].rearrange("p (r w) -> p r w", w=W)
                if t == 0:
                    nc_.scalar.activation(dst3, src3, ACTF.Identity,
                                          bias=vecst[:, 1:2])
                else:
                    nc_.vector.tensor_scalar(dst3, src3, vecst[:, 2:3], None,
                                             op0=ALU.add)
            # xg band remaps: band b needs x3sf rows 14b..14b+16, i.e. conv3
            # chunks up to ceil((14b+15)/8)-1 = [1,3,5,6][b]
            for b in range(NB):
                if band_last_chunk[b] == c:
                    for t in range(2):
                        psl = slice(32 * b + 16 * t, 32 * b + 16 * t + 16)
                        # src [128=(g,s), 16, 58] pairs with dst [16, 8, 16, 58]
                        # in flat element order (sizes match; no partition split)
                        # sync queue: keeps the scalar FIFO free for phase-C ACTs
                        nc.sync.dma_start(
                            xgA[psl],
                            x3sf[:, t, 14 * b:14 * b + 16, :])
                    # xgB band: xgA shifted one element right (flat, partition-
                    # local) for dw=0 4B alignment — per band so it lands early
                    bsl = slice(32 * b, 32 * b + 32)
                    nc.sync.dma_start(
                        xgB[bsl].rearrange("p s r w -> p (s r w)")[:, 1:],
                        xgA[bsl].rearrange("p s r w -> p (s r w)")
                        [:, 0:SHARE * 16 * 58 - 1])
        pscope1.close()

        xg = {-1: xgA, 0: xgB, 1: xgA}
        xgo = {-1: 0, 0: 2, 1: 2}   # col offset of the dh-window per dw
        x2 = {-1: x2b, 0: x2bB, 1: x2b}
        x2o = {-1: 0, 0: 2, 1: 2}

        # ---- phase C: per-k logits + exp + Z (PE bf16 + ACT)
        frpool = ctx.enter_context(tc.tile_pool(name="frpool", bufs=4))
        hpool = ctx.enter_context(tc.tile_pool(name="hpool", bufs=3))
        pscope2 = ExitStack()
        ppz = pscope2.enter_context(tc.tile_pool(name="ppz", bufs=1, space="PSUM"))
        pscope3 = ExitStack()
        pph = pscope3.enter_context(tc.tile_pool(name="pph", bufs=3, space="PSUM"))
        ek = {}
        zps = ppz.tile([128, 1024], F32, tag="zps")
        wsl = [slice(0, 512), slice(512, Q)]
        # software-pipelined: iteration i emits stage-1 work for k_i and
        # stage-2 work for k_{i-1}, so no engine FIFO head-of-line blocks.
        hp_d = {}
        hps_d = {}
        wps_d = {}
        for i in range(10):
            if i >= 1:
                kp = OFFS_ORD[i - 1]
                # finish hp(k_{i-1}) on DVE ([512:784] half)
                nc_.vector.tensor_scalar(hp_d[kp][:, 512:Q],
                                         hps_d[kp][:, 512:Q],
                                         vecst[:, 3:4], 0.0,
                                         op0=ALU.add, op1=ALU.max)
            if i < 9:
                k = OFFS_ORD[i]
                dh, dw = OFFS[k]
                co_ = x2o[dw]
                fs = frpool.tile([128, BH, W], BF16, tag="fs")
                nc_.vector.tensor_tensor(
                    fs[:], x1b[:].rearrange("p (r w) -> p r w", w=W),
                    x2[dw][:, 1 + dh:1 + dh + BH, co_:co_ + W],
                    ALU.subtract)
                fr = frpool.tile([128, Q], BF16, tag="fr")
                nc_.vector.tensor_scalar(fr[:].rearrange("p (r w) -> p r w", w=W),
                                         fs[:], 0.0, None, op0=ALU.max)
                hps = pph.tile([128, 1024], F32, tag="hw")
                hps_d[k] = hps
                # same-stationary matmuls back-to-back (one weight load/pair)
                for s in wsl:
                    nc_.tensor.matmul(hps[:, s], w1t[:], fr[:, s],
                                      start=True, stop=False)
                for s in wsl:
                    nc_.tensor.matmul(hps[:, s], post[:], prt[0:8, k, s],
                                      start=False, stop=True)
                hp = hpool.tile([128, Q], BF16, tag="hp")
                hp_d[k] = hp
                nc_.scalar.activation(hp[:, 0:512], hps[:, 0:512], ACTF.Relu,
                                      bias=vecst[:, 3:4])
            if i >= 1:
                kp = OFFS_ORD[i - 1]
                wps = pph.tile([128, 1024], F32, tag="hw")
                wps_d[kp] = wps
                for s in wsl:
                    nc_.tensor.matmul(wps[:, s], w2t[:], hp_d[kp][:, s],
                                      start=True, stop=True)
                e = epool.tile([128, Q], BF16, tag="e")
                nc_.scalar.activation(e[:], wps[:, 0:Q], ACTF.Exp,
                                      bias=vecst[:, 4:5])
                for s in wsl:
                    nc_.tensor.matmul(zps[:, s], identt[:], e[:, s],
                                      start=(i == 1), stop=(i == 9))
                ek[kp] = e
        pscope3.close()

        # ---- phase E: aggregation + convo, per half-band row group qp.
        # The rz = exp(-ln(Z)) computation (and its ACT table swaps) is
        # emitted AFTER the first quad's multiply/matmul rounds so the two
        # table loads hide under quad-0 compute.
        rzpool = ctx.enter_context(tc.tile_pool(name="rzpool", bufs=1))
        lnz = rzpool.tile([128, Q], F32, tag="lnz")
        rz16 = rzpool.tile([128, Q], BF16, tag="rz16")
        pkpool = ctx.enter_context(tc.tile_pool(name="pkpool", bufs=4))
        sqf = sbp.tile([128, SHARE, Q], BF16, tag="sqf")
        lkpool = ctx.enter_context(tc.tile_pool(name="lkpool", bufs=3))
        opool = ctx.enter_context(tc.tile_pool(name="opool", bufs=4))
        samsb = sbp.tile([128, 2, NPIX], BF16, tag="samsb")
        pscope4 = ExitStack()
        pps = pscope4.enter_context(tc.tile_pool(name="pps", bufs=1, space="PSUM"))
        ppo = pscope4.enter_context(tc.tile_pool(name="ppo", bufs=2, space="PSUM"))

        first_quad = [True]

        for qp in range(2):
            for sq in range(2):
                sam = pps.tile([128, 2048], F32, tag="sam")
                samv = sam[:].rearrange("p (a j) -> p a j", j=512)[:, :, 0:HBW]
                for j, k in enumerate(OFFS_ORD):
                    dh, dw = OFFS[k]
                    pk = pkpool.tile([128, NSQ, 7, W], BF16, tag="pk")
                    co_ = xgo[dw]
                    r0 = 1 + dh + 7 * qp
                    nc_.vector.tensor_tensor(
                        pk[:],
                        xg[dw][:, NSQ * sq:NSQ * (sq + 1), r0:r0 + 7,
                               co_:co_ + W],
                        ek[k][:].rearrange("p (r w) -> p r w", w=W)
                        [:, 7 * qp:7 * qp + 7, :].unsqueeze(1)
                        .broadcast_to((128, NSQ, 7, W)),
                        ALU.mult)
                    pkf = pk[:].rearrange("p a r w -> p (a r w)")
                    for c4 in range(4):
                        nc_.tensor.matmul(
                            sam[:, 512 * c4:512 * c4 + HBW], identt[:],
                            pkf[:, HBW * c4:HBW * (c4 + 1)],
                            start=(j == 0), stop=(j == 8))
                if first_quad[0]:
                    # rz = exp(-ln(Z)): table loads overlap quad-0 compute
                    nc_.scalar.activation(lnz[:], zps[:, 0:Q], ACTF.Ln)
                    nc_.scalar.activation(rz16[:], lnz[:], ACTF.Exp,
                                          scale=-1.0)
                    first_quad[0] = False
                # leaky(sam) then * rz -> sqfull quad slice
                lk = lkpool.tile([128, NSQ, HBW], BF16, tag="lk")
                nc_.scalar.activation(lk[:], samv, ACTF.Prelu, alpha=NEG)
                nc_.vector.tensor_tensor(
                    sqf[:, NSQ * sq:NSQ * (sq + 1), HBW * qp:HBW * (qp + 1)],
                    lk[:],
                    rz16[:, HBW * qp:HBW * (qp + 1)].unsqueeze(1)
                    .broadcast_to((128, NSQ, HBW)),
                    ALU.mult)
            # sam remap: (b, g) partitions -> channel partitions, per (t, b)
            for t in range(2):
                for b in range(NB):
                    # dst [128=(g,s), 392] pairs with src [16, 8, 392] in flat
                    # element order (no partition split on either side)
                    nc.sync.dma_start(
                        samsb[:, t, Q * b + HBW * qp:Q * b + HBW * (qp + 1)],
                        sqf[32 * b + 16 * t:32 * b + 16 * t + 16, :,
                            HBW * qp:HBW * (qp + 1)])
            # convo on this qp's half-bands
            for b in range(NB):
                po = Q * b + HBW * qp
                for to in range(2):
                    pso = ppo.tile([128, HBW], F32, tag="pso")
                    nc_.tensor.matmul(pso[:], cot[:, 0, 128 * to:128 * (to + 1)],
                                      samsb[:, 0, po:po + HBW],
                                      start=True, stop=False)
                    nc_.tensor.matmul(pso[:], cot[:, 1, 128 * to:128 * (to + 1)],
                                      samsb[:, 1, po:po + HBW],
                                      start=False, stop=True)
                    o = opool.tile([128, HBW], F32, tag="o")
                    nc_.scalar.activation(o[:], pso[:], ACTF.Prelu,
                                          bias=vecst[:, 5 + to:6 + to],
                                          alpha=NEG)
                    o2 = opool.tile([128, HBW], F32, tag="o2")
                    nc_.vector.tensor_tensor(
                        o2[:], o[:], xfull[:, to, po:po + HBW].bitcast(F32),
                        ALU.add)
                    nc.sync.dma_start(
                        outd[128 * to:128 * (to + 1), po:po + HBW], o2[:])
        pscope4.close()
        pscope2.close()

        if DEBUG:
            nc.sync.dma_start(dbg["d_x12s"][:], x12s[:])
            nc.sync.dma_start(dbg["d_x1b"][:], x1b[:])
            nc.sync.dma_start(dbg["d_x2b"][:],
                              x2b[:].rearrange("p r w -> p (r w)"))
            nc.sync.dma_start(dbg["d_e0"][:], ek[0][:])
            nc.sync.dma_start(dbg["d_e4"][:], ek[4][:])
            nc.sync.dma_start(dbg["d_rz"][:], rz16[:])
            nc.sync.dma_start(dbg["d_sqf"][:],
                              sqf[:].rearrange("p s q -> p (s q)"))
            nc.sync.dma_start(dbg["d_samsb"][:],
                              samsb[:].rearrange("p t q -> p (t q)"))
            nc.sync.dma_start(dbg["d_xga"][:],
                              xgA[:].rearrange("p s r w -> p (s r w)"))

    nc.compile()
    return nc


# --------------------------------------------------------------- entrypoint
def _get_program():
    if "nc" not in _CACHE:
        _CACHE["nc"] = _build_program()
    return _CACHE["nc"]


def _run(inputs, trace):
    inputs = {k: np.asarray(v) for k, v in inputs.items()}
    consts = _host_consts(inputs)
    nc = _get_program()
    x = inputs["x"].reshape(B, CIN, NPIX).astype(np.float32)
    in_maps = []
    for b in range(B):
        m = {k: v for k, v in consts.items()}
        m["xin"] = x[b]
        in_maps.append(m)
    res = run_bass_kernel_spmd(nc, in_maps, list(range(B)), trace=trace)
    out = np.stack([res.results[i]["out"] for i in range(B)])
    return out.reshape(B, CIN, H, W).astype(np.float32), res


def kernel(**inputs):
    return _run(inputs, False)[0]


def kernel_traced(**inputs):
    """Like kernel() but with NTFF tracing; returns (out, BassKernelResults)."""
    return _run(inputs, True)

